# revision 99
# baseline (speedup 1.0000x reference)
"""Trainium2 Bass kernel for nn_CellularAutomatonDecoder.

Model (per reference):
  cells = embed[tokens] + pos_embed                        (B, T, D)
  rule_bias MLP from mean(c_states); const_bias = rule_bias @ W1b + b1
  8x CA steps: pre = cells@W1c + roll(cells,+1)@W1l + roll(cells,-1)@W1r + cb
               cells = a*cells + (1-a)*tanh(gelu(pre) @ W2 + b2)
  out = LN(cells) @ head_w                                 (B, T, V)

Sharding: pure data-parallel over batch across 8 cores (256 rows each).

Device design notes (v2 — fp8 DoubleRow + pipelined tail):
- feature-major state sigma[d=128, 8192] f32 in SBUF, t-major columns
  (col j = t*256 + b): the T-roll is a +-256 column shift.
- step modes "fffffttb": steps 0-4 full-fp8 (taps W1l/W1r and W2 as
  e4m3 DoubleRow pairs, center tap f32r), steps 5-6 tap-only-fp8
  (W2/gelu output stay f32), step 7 all-bf16 taps. Early steps carry the
  least error amplification, so fp8 goes there. PE per step drops from
  ~27.3us (f32r) to ~18.4us (full fp8) / ~21.4us (tap-fp8).
- fp8 shadow sig8 = 16*sigma with a 256-col halo on both sides so a
  DoubleRow moving operand is a plain contiguous slice:
  sig8[:, c0:c0+1024].rearrange("p (two n) -> p two n") pairs
  (sigma[j-256], sigma[j+256]) for output cols j in [c0, c0+512).
- scales: f32r/bf16-packed W1c x 32768, fp8 weights x 2048, sig8/h8 x 16
  -> every accumulating matmul lands in PSUM at x32768; descale via the
  activation scale argument (gelu scale=ia/32768, tanh scale=1/32768).
- token gather: bf16 tokens broadcast via K=1 ones-matmul (no 4MB DMA,
  warms the PE/HAM from ~1us), one-hot compares on DVE, embed via bf16
  one-hot matmul, pos added by the ACT copy out of PSUM.
- blends lag the chunk matmuls by 2 across ALL steps (no step barrier).
- v3/v4 session updates: step 7 runs fp8 taps too (modes fffffttt);
  tail sq + PSUM->SBUF output drains run on DVE for the first 3 tails
  (step-7 gelu/tanh still owns ACT) and on ACT for the last 5 (ACT is
  idle there, DVE is the tail bottleneck); single fused 2048-col h8
  cast; quake-rsqrt batched per chunk-pair from an SBUF stats copy
  (frees the shared "new" PSUM tag fast); o_t/out in bf16 (host casts
  back); ALL input DMAs issue from the sync queue so the in-order ACT
  queue never parks DMA descriptors in front of the init drains.
- tail: the final blend writes bf16 sigma
  directly; LN sums come token-major from N=1 ones-matmuls that reuse
  the head's sigma-block LDWEIGHTS (no PE micro-transposes, no [1,N]
  row copies); per-chunk inv-std math feeds the head scale; output DMAs
  start per 4-block group. PE stays dense to the end so HAM never
  re-throttles (the old kernel ran its last 50us at half clock).
"""

import os
import sys

import numpy as np

for _p in ("/opt/trn_rl_repo", "/root/.axon_site/_ro/trn_rl_repo"):
    if os.path.isdir(_p) and _p not in sys.path:
        sys.path.append(_p)

from contextlib import ExitStack

import ml_dtypes

import concourse.bacc as bacc
import concourse.tile as tile
from concourse import mybir
from concourse.bass_utils import run_bass_kernel_spmd

F32 = mybir.dt.float32
F32R = mybir.dt.float32r
BF16 = mybir.dt.bfloat16
FP8 = mybir.dt.float8e4
AF = mybir.ActivationFunctionType
ALU = mybir.AluOpType
AX = mybir.AxisListType
DR = mybir.MatmulPerfMode.DoubleRow

B, T, D, V, CDIM = 2048, 32, 128, 256, 128
NEV = 8
EPS = 1e-5
NC = 8
BL = B // NC          # 256 batch rows per core
NTOK = BL * T         # 8192 tokens per core
CH = 1024             # token chunk (columns)
NCH = NTOK // CH      # 8 chunks
NBLK = NTOK // 128    # 64 head blocks

SC = 2048.0           # f32r center-tap / fp8 weight / PSUM scale
SX = 16.0             # fp8 h-activation extra scale
MODES = "fffffttt"    # per-step: f=full fp8, t=tap-only fp8, b=bf16

TRACE = False
_CACHE = {}


def _pieces(dst0, n, shift, maxlen=512):
    """Contiguous (dst, src, len) pieces of src = (dst + shift) mod NTOK."""
    out = []
    j = 0
    while j < n:
        s = (dst0 + j + shift) % NTOK
        ln = min(n - j, NTOK - s, maxlen)
        out.append((dst0 + j, s, ln))
        j += ln
    return out


def _build(a, has_lnb):
    ia = 1.0 - a
    nc = bacc.Bacc("TRN2", target_bir_lowering=False, debug=False, num_devices=NC)

    tok_d = nc.dram_tensor("tok", [1, NTOK], BF16, kind="ExternalInput").ap()
    ones_d = nc.dram_tensor("onesb", [128, 4], BF16, kind="ExternalInput").ap()
    cpack_d = nc.dram_tensor("cpack", [128, 44], F32, kind="ExternalInput").ap()
    epack_d = nc.dram_tensor("epack", [128, 256], BF16, kind="ExternalInput").ap()
    wca_d = nc.dram_tensor("wca", [128, 256], F32R, kind="ExternalInput").ap()
    w2f_d = nc.dram_tensor("w2f", [128, 256], F32R, kind="ExternalInput").ap()
    wb_d = nc.dram_tensor("wb", [128, 1280], BF16, kind="ExternalInput").ap()
    w8_d = nc.dram_tensor("w8", [128, 768], FP8, kind="ExternalInput").ap()
    fpack_d = nc.dram_tensor("fpack", [128, 768], F32, kind="ExternalInput").ap()

    out_d = nc.dram_tensor("out", [NTOK, V], BF16, kind="ExternalOutput").ap()
    out_r = out_d.rearrange("(b t) v -> b t v", t=T)
    # [b, h, t, v] view: lets one DMA cover both 128-row halves of a chunk
    out_m = out_d.rearrange("(h b t) v -> b h t v", h=2, t=T)

    with tile.TileContext(nc) as tc, ExitStack() as ctx:
        # ---- persistent SBUF ----
        # DMA priority: tokens/cpack/epack feed init; w8/wca feed the first
        # fp8 steps; wb (step 7 weights) can land last.
        wpool = ctx.enter_context(tc.tile_pool(name="weights", bufs=1))
        tokbc = wpool.tile([128, NTOK], BF16, tag="tokbc")

        def tokdma(eng, g):
            src = tok_d[0:1, g * CH:(g + 1) * CH].broadcast_to((128, CH))
            eng.dma_start(tokbc[:, g * CH:(g + 1) * CH], src)

        epack = wpool.tile([128, 256], BF16, tag="epack")
        cpack = wpool.tile([128, 44], F32, tag="cpack")
        onesb = wpool.tile([128, 4], BF16, tag="onesb")
        vidb = onesb[:, 2:4]
        wca = wpool.tile([128, 256], F32R, tag="wca")
        w2f = wpool.tile([128, 256], F32R, tag="w2f")
        wb = wpool.tile([128, 1280], BF16, tag="wb")
        w8 = wpool.tile([128, 768], FP8, tag="w8")
        fpack = wpool.tile([128, 768], F32, tag="fpack")

        # ALL input DMAs issue from the sync queue: the scalar (ACT) queue is
        # in-order, and DMA issues parked there block the init PSUM drains
        # for ~8us, which idles the PE long enough for HAM to re-throttle
        # The 4 critical early DMAs issue from the SCALAR queue: its boot
        # ends ~1us before sync's and its first ACTIVATE isn't needed until
        # ~12us, so cpack/tok0/epack land ~2us earlier and the init chain
        # (one-hot -> embed matmul -> drain) starts sooner. Everything else
        # stays on sync so the ACT queue is clear well before the drains.
        nc.scalar.dma_start(cpack[:], cpack_d)
        tokdma(nc.scalar, 0)
        nc.scalar.dma_start(epack[:], epack_d)
        nc.scalar.dma_start(fpack[:], fpack_d)
        tokdma(nc.sync, 1)
        tokdma(nc.sync, 2)
        nc.sync.dma_start(w8[:], w8_d)
        nc.sync.dma_start(wca[:], wca_d)
        tokdma(nc.sync, 3)
        tokdma(nc.sync, 4)
        tokdma(nc.sync, 5)
        tokdma(nc.sync, 6)
        tokdma(nc.sync, 7)
        nc.sync.dma_start(w2f[:], w2f_d)
        nc.sync.dma_start(onesb[:], ones_d)
        nc.sync.dma_start(wb[:], wb_d)

        posT_s, cT_s = cpack[:, 0:32], cpack[:, 32:36]
        bc1_s, bc2_s = cpack[:, 36:38], cpack[:, 38:39]
        b1_s, b2_s = cpack[:, 39:41], cpack[:, 41:42]
        vid_s = cpack[:, 42:44]
        wcb, wlb, wrb = wb[:, 0:256], wb[:, 256:512], wb[:, 512:768]
        w2b, hwcb = wb[:, 768:1024], wb[:, 1024:1280]
        w1b_s, wc1_s, wc2_s = fpack[:, 0:256], fpack[:, 256:512], fpack[:, 512:768]

        spool = ctx.enter_context(tc.tile_pool(name="state", bufs=1))
        sig = spool.tile([128, NTOK], F32R, tag="sigma")
        sig8 = spool.tile([128, NTOK + 512], FP8, tag="sig8")
        sigb = spool.tile([128, NTOK], BF16, tag="sigb")

        mlp_sb = ctx.enter_context(tc.tile_pool(name="mlp_sb", bufs=1))
        cbias_s = mlp_sb.tile([128, 2], F32, tag="cbias")

        # shared pools, all phases (no release barriers)
        pp = ctx.enter_context(tc.tile_pool(name="psum", bufs=1, space="PSUM"))
        # one merged work pool: per-tag bufs set at tile() call sites; fewer
        # pools -> fewer framework sync structures in the exit drain
        sbw = ctx.enter_context(tc.tile_pool(name="work_sb", bufs=2))
        sbhb = sbh8 = sbhf = sbt = sbsq = sbst = sbo = sbw

        def pre_t(name, cols=CH):
            return pp.tile([128, cols], F32, tag="pre", name=name, bufs=3)

        def new_t(shape, name):
            return pp.tile(shape, F32, tag="new", name=name, bufs=1)

        # ---- PE warmup: ~5us of dense full-K matmuls so HAM unthrottles
        # before the real init stream. Source is a memset tile, not a DMA'd
        # one, so the dummies start during the input DMAs (~6us earlier).
        wdum = wpool.tile([128, 256], BF16, tag="wdum")
        nc.vector.memset(wdum[:], 1.0)
        warm_ps = new_t([128, 512], "warm_ps")
        for wi in range(40):
            nc.tensor.matmul(warm_ps[:, 0:256], wdum[:, 0:128], wdum[:],
                             start=True, stop=True)

        # ---- init: one-hot on DVE (4x bf16 tier), embed matmul ----
        # (the rule-bias MLP is emitted AFTER the init loop: its matmuls wait
        # on late DMAs + a cross-engine chain, and at the head of the in-order
        # PE queue they stall the embed matmuls long enough for HAM to drop
        # the PE to half clock for the whole init)
        for ci in range(NCH):
            c0 = ci * CH
            oh = sbhb.tile([128, 2 * CH], BF16, tag="hb", name="oh")
            nc.vector.tensor_scalar(oh[:, 0:CH], tokbc[:, c0:c0 + CH],
                                    vid_s[:, 0:1], None, ALU.is_equal)
            nc.vector.tensor_scalar(oh[:, CH:2 * CH], tokbc[:, c0:c0 + CH],
                                    vid_s[:, 1:2], None, ALU.is_equal)
            cells_ps = pre_t("cells_ps")
            for k in range(2):
                jc = slice(k * 512, (k + 1) * 512)
                nc.tensor.matmul(cells_ps[:, jc], epack[:, 0:128],
                                 oh[:, k * 512:(k + 1) * 512],
                                 start=True, stop=False)
                nc.tensor.matmul(cells_ps[:, jc], epack[:, 128:256],
                                 oh[:, CH + k * 512:CH + (k + 1) * 512],
                                 start=False, stop=True)
            # pos_embed rides the ACT drain bias: init is PE-bound at cold
            # clock, so keep matmuls minimal and let the (starved) ACT do pos
            for kb in range(CH // 256):
                tt = (c0 + kb * 256) // 256
                nc.scalar.activation(sig[:, c0 + kb * 256: c0 + (kb + 1) * 256],
                                     cells_ps[:, kb * 256:(kb + 1) * 256],
                                     AF.Identity, bias=posT_s[:, tt:tt + 1])
            nc.vector.tensor_copy(sig8[:, 256 + c0:256 + c0 + CH],
                                   sig[:, c0:c0 + CH])
            if ci == NCH - 1:
                nc.vector.tensor_copy(sig8[:, 0:256], sig[:, NTOK - 256:NTOK])
            if ci == 0:
                nc.vector.tensor_copy(sig8[:, 256 + NTOK:512 + NTOK],
                                      sig[:, 0:256])

        # ---- rule-bias MLP (tiny; overlaps the init drains) ----
        cp_s = mlp_sb.tile([128, 1], F32, tag="cp")
        nc.vector.tensor_reduce(cp_s[:], cT_s[:], axis=AX.X, op=ALU.add)
        y1_ps = new_t([128, 2], "y1_ps")
        for h in range(2):
            nc.tensor.matmul(y1_ps[:, h:h + 1], wc1_s[:, h * 128:(h + 1) * 128],
                             cp_s[:], start=True, stop=True)
        y1g_s = mlp_sb.tile([128, 2], F32, tag="y1g")
        for h in range(2):
            nc.scalar.activation(y1g_s[:, h:h + 1], y1_ps[:, h:h + 1], AF.Gelu,
                                 bias=bc1_s[:, h:h + 1], scale=0.25)
        rb_ps = new_t([128, 2], "rb_ps")
        nc.tensor.matmul(rb_ps[:, 0:1], wc2_s[:, 0:128], y1g_s[:, 0:1],
                         start=True, stop=False)
        nc.tensor.matmul(rb_ps[:, 0:1], wc2_s[:, 128:256], y1g_s[:, 1:2],
                         start=False, stop=True)
        rb_s = mlp_sb.tile([128, 1], F32, tag="rb")
        nc.scalar.activation(rb_s[:], rb_ps[:, 0:1], AF.Identity, bias=bc2_s[:, 0:1])
        cb_ps = new_t([128, 2], "cb_ps")
        for h in range(2):
            nc.tensor.matmul(cb_ps[:, h:h + 1], w1b_s[:, h * 128:(h + 1) * 128],
                             rb_s[:], start=True, stop=True)
        for h in range(2):
            nc.scalar.activation(cbias_s[:, h:h + 1], cb_ps[:, h:h + 1], AF.Identity,
                                 bias=b1_s[:, h:h + 1])

        # ---- evolve ----
        w8r = w8[:].rearrange("p (x m) -> p x m", m=128)  # x: wl0,wr0,wl1,wr1,w20,w21
        sig8r = sig8  # halo offset: sigma col c maps to sig8 col c+256

        def emit_fp8_pre(ci):
            c0 = ci * CH
            pre = [pre_t(f"pre{h}") for h in range(2)]
            for h in range(2):
                hcols = slice(h * 128, (h + 1) * 128)
                lhs8 = w8[:, h * 256:(h + 1) * 256].rearrange(
                    "p (two m) -> p two m", two=2)
                for k in range(2):
                    c0k = c0 + k * 512
                    jc = slice(k * 512, (k + 1) * 512)
                    nc.tensor.matmul(pre[h][:, jc], wca[:, hcols],
                                     sig[:, c0k:c0k + 512], start=True, stop=False)
                    rhs8 = sig8r[:, c0k:c0k + 1024].rearrange(
                        "p (two n) -> p two n", two=2)
                    nc.tensor.matmul(pre[h][:, jc], lhs8, rhs8,
                                     start=False, stop=True, perf_mode=DR)
            return pre

        def emit_stage1(s, ci, mode):
            """pre matmuls + gelu (+ fp8 cast of h). Returns h for stage 2.

            (Pool-engine blend offload was tried and reverted: the Pool SBUF
            port is shared with the DVE, so pool elementwise ops serialize
            against DVE streaming instead of overlapping.)
            """
            us = 1.0
            c0 = ci * CH
            if mode in ("f", "t"):
                pre = emit_fp8_pre(ci)
            else:  # 'b': bf16 taps from sigb
                pre = [pre_t(f"pre{h}") for h in range(2)]
                for h in range(2):
                    hcols = slice(h * 128, (h + 1) * 128)
                    for k in range(2):
                        jc = slice(k * 512, (k + 1) * 512)
                        nc.tensor.matmul(pre[h][:, jc], wcb[:, hcols],
                                         sigb[:, c0 + k * 512:c0 + (k + 1) * 512],
                                         start=True, stop=False)
                    for dd, ss, ll in _pieces(c0, CH, -256):
                        nc.tensor.matmul(pre[h][:, dd - c0:dd - c0 + ll],
                                         wlb[:, hcols], sigb[:, ss:ss + ll],
                                         start=False, stop=False)
                    for dd, ss, ll in _pieces(c0, CH, +256):
                        nc.tensor.matmul(pre[h][:, dd - c0:dd - c0 + ll],
                                         wrb[:, hcols], sigb[:, ss:ss + ll],
                                         start=False, stop=True)
            if mode == "f":
                h_b = sbhb.tile([128, 2 * CH], BF16, tag="hb", name="h_b")
                for h in range(2):
                    nc.scalar.activation(h_b[:, h * CH:(h + 1) * CH], pre[h][:],
                                         AF.Gelu, bias=cbias_s[:, h:h + 1],
                                         scale=ia * us / SC)
                h_8 = sbh8.tile([128, 2 * CH], FP8, tag="h8", name="h_8")
                nc.vector.tensor_scalar(h_8[:], h_b[:], SX, None, ALU.mult)
                return h_8
            elif mode == "t":
                h_f = [sbhf.tile([128, CH], F32R, tag="hf", name=f"hf{h}", bufs=4)
                       for h in range(2)]
                for h in range(2):
                    nc.scalar.activation(h_f[h][:], pre[h][:], AF.Gelu,
                                         bias=cbias_s[:, h:h + 1],
                                         scale=ia * us / SC)
                return h_f
            else:
                h_b = sbhb.tile([128, 2 * CH], BF16, tag="hb", name="h_b")
                for h in range(2):
                    nc.scalar.activation(h_b[:, h * CH:(h + 1) * CH], pre[h][:],
                                         AF.Gelu, bias=cbias_s[:, h:h + 1],
                                         scale=ia * us)
                return h_b

        def emit_stage2(s, ci, mode, hin):
            """W2 matmuls + tanh -> t tile (pre-scaled by a^-(s+1) so the
            blend is a plain add in u-space; the last step skips the scale
            and blends via one DVE STT instead)."""
            new_ps = new_t([128, CH], "new_ps")
            if mode == "f":
                lhsw2 = w8[:, 512:768].rearrange("p (two m) -> p two m", two=2)
                h8r = hin[:].rearrange("p (two n) -> p two n", two=2)
                for k in range(2):
                    jc = slice(k * 512, (k + 1) * 512)
                    nc.tensor.matmul(new_ps[:, jc], lhsw2, h8r[:, :, jc],
                                     start=True, stop=True, perf_mode=DR)
                tanh_scale = 1.0 / (SC * SX)
            elif mode == "t":
                for k in range(2):
                    jc = slice(k * 512, (k + 1) * 512)
                    nc.tensor.matmul(new_ps[:, jc], w2f[:, 0:128], hin[0][:, jc],
                                     start=True, stop=False)
                    nc.tensor.matmul(new_ps[:, jc], w2f[:, 128:256], hin[1][:, jc],
                                     start=False, stop=True)
                tanh_scale = 1.0
            else:
                for k in range(2):
                    jc = slice(k * 512, (k + 1) * 512)
                    nc.tensor.matmul(new_ps[:, jc], w2b[:, 0:128],
                                     hin[:, k * 512:(k + 1) * 512],
                                     start=True, stop=False)
                    nc.tensor.matmul(new_ps[:, jc], w2b[:, 128:256],
                                     hin[:, CH + k * 512:CH + (k + 1) * 512],
                                     start=False, stop=True)
                tanh_scale = 1.0
            t_t = sbt.tile([128, CH], F32, tag="t", name="t_t", bufs=4)
            nc.scalar.activation(t_t[:], new_ps[:], AF.Tanh, bias=b2_s[:, 0:1],
                                 scale=tanh_scale)
            return t_t

        def emit_blend(s, ci, t_t):
            c0 = ci * CH
            if s == NEV - 1:
                # final blend: write bf16 state for stats + head
                nc.vector.scalar_tensor_tensor(
                    sigb[:, c0:c0 + CH], sig[:, c0:c0 + CH], a, t_t[:],
                    op0=ALU.mult, op1=ALU.add)
                return
            nc.vector.scalar_tensor_tensor(
                sig[:, c0:c0 + CH], sig[:, c0:c0 + CH], a, t_t[:],
                op0=ALU.mult, op1=ALU.add)

        def emit_post(s, ci):
            # fp8 shadow refresh, emitted one pipeline slot after the pool
            # add so the in-order DVE queue never blocks on the Pool engine
            c0 = ci * CH
            if s == NEV - 1:
                emit_tail(ci)
                return
            nc.vector.tensor_copy(sig8[:, 256 + c0:256 + c0 + CH],
                                  sig[:, c0:c0 + CH])
            if ci == NCH - 1:
                nc.vector.tensor_copy(sig8[:, 0:256],
                                      sig[:, NTOK - 256:NTOK])
            if ci == 0:
                nc.vector.tensor_copy(sig8[:, 256 + NTOK:512 + NTOK],
                                      sig[:, 0:256])

        tail_st = {}   # ci -> (slot in stpair, o_ts)
        tail_pend = []

        def emit_tail_a(ci, stpair, o_pair, slot, use_act):
            # per-chunk tail: sq, stats matmuls (copied to SBUF right away so
            # the shared "new" PSUM tag frees fast), head matmuls + bf16 drain
            c0 = ci * CH
            # x^2: DVE while step-7 still owns ACT (early tails); ACT
            # (Square, same table set) for the late tails once it idles
            sq = sbsq.tile([128, CH], BF16, tag="sq", name="sq")
            if use_act:
                nc.scalar.activation(sq[:], sigb[:, c0:c0 + CH], AF.Square)
            else:
                nc.vector.tensor_mul(sq[:], sigb[:, c0:c0 + CH],
                                     sigb[:, c0:c0 + CH])
            st_ps = new_t([128, 16], "st_ps")
            for j in range(8):
                blk = 8 * ci + 2 * (j % 4) + (j // 4)  # order: hh-major groups
                bc = blk * 128
                nc.tensor.matmul(st_ps[:, 2 * j:2 * j + 1],
                                 sigb[:, bc:bc + 128], onesb[:, 0:1],
                                 start=True, stop=True)
                nc.tensor.matmul(st_ps[:, 2 * j + 1:2 * j + 2],
                                 sq[:, bc - c0:bc - c0 + 128], onesb[:, 0:1],
                                 start=True, stop=True)
            if use_act:
                nc.scalar.activation(stpair[:, 16 * slot:16 * slot + 16],
                                     st_ps[:], AF.Identity)
            else:
                nc.vector.tensor_copy(stpair[:, 16 * slot:16 * slot + 16],
                                      st_ps[:])
            # head matmuls drain unscaled to bf16 right away (PSUM freed
            # fast, no stats dependency); inv-std scale applied in place
            t0 = 4 * ci
            for hh in range(2):
                ap_ = pre_t(f"a{hh}", cols=CH)
                for tl in range(4):
                    blk = 2 * (t0 + tl) + hh
                    nc.tensor.matmul(ap_[:, tl * 256:(tl + 1) * 256],
                                     sigb[:, blk * 128:(blk + 1) * 128],
                                     hwcb[:], start=True, stop=True)
                oc = hh * 2 * CH + slot * CH
                if use_act:
                    nc.scalar.activation(o_pair[:, oc:oc + CH], ap_[:],
                                         AF.Identity)
                else:
                    nc.vector.tensor_copy(o_pair[:, oc:oc + CH], ap_[:])

        def emit_tail_b(pair):
            # per-pair: one batched quake-rsqrt over both chunks' stats
            # (halves the small-op DVE overhead), then scales + out DMAs
            (c1, st1, op1), (c2, st2, op2) = pair
            stpair, o_pair = st1, op1
            g = nc.vector
            st3 = stpair[:].rearrange("p (b two) -> p b two", two=2)
            s1ap, s2ap = st3[:, :, 0], st3[:, :, 1]
            # rescaled variance: work on v' = s2 - s1^2/128 + 128*EPS/ia^2;
            # the constant sqrt(128)/ia is folded into the head weights, so
            # rsqrt(v') is the right scale directly (one DVE op fewer)
            m2 = sbst.tile([128, 16], F32, tag="m2", name="m2", bufs=3)
            g.tensor_mul(m2[:], s1ap, s1ap)
            vf = sbst.tile([128, 16], F32, tag="vf", name="vf", bufs=3)
            g.scalar_tensor_tensor(vf[:], m2[:], -1.0 / 128.0, s2ap,
                                   op0=ALU.mult, op1=ALU.add)
            g.tensor_scalar_add(vf[:], vf[:], float(128.0 * EPS / (ia * ia)))
            us = sbst.tile([128, 16], F32, tag="m2", name="us", bufs=3)
            g.tensor_scalar(us[:].bitcast(mybir.dt.uint32),
                            vf[:].bitcast(mybir.dt.uint32),
                            1, None, ALU.logical_shift_right)
            usf = sbst.tile([128, 16], F32, tag="m2", name="usf", bufs=3)
            g.tensor_copy(usf[:], us[:].bitcast(mybir.dt.uint32))
            yf = sbst.tile([128, 16], F32, tag="m2", name="yf", bufs=3)
            g.tensor_scalar(yf[:], usf[:], -1.0, float(0x5F375A86),
                            ALU.mult, op1=ALU.add)
            y0 = sbst.tile([128, 16], F32, tag="m2", name="y0", bufs=3)
            g.tensor_copy(y0[:].bitcast(mybir.dt.uint32), yf[:])
            yy = y0
            for it in range(1):
                r = sbst.tile([128, 16], F32, tag="vf", name=f"r{it}", bufs=3)
                g.tensor_mul(r[:], vf[:], yy[:])
                g.tensor_mul(r[:], r[:], yy[:])
                w_ = sbst.tile([128, 16], F32, tag="vf", name=f"w{it}", bufs=3)
                g.tensor_scalar(w_[:], r[:], -0.5, 1.5, ALU.mult, op1=ALU.add)
                yn = sbst.tile([128, 16], F32, tag="inv" if it == 0 else "m2",
                               name=f"y{it + 1}")
                g.tensor_mul(yn[:], yy[:], w_[:])
                yy = yn
            inv = yy
            last = tail_st.get("n", 0) == 2 * (NCH // 2)
            opr = o_pair[:].rearrange("p (h t v) -> p h t v", h=2, t=8)
            for slot, ci in enumerate((c1, c2)):
                for hh in range(2):
                    for tl in range(4):
                        j = 8 * slot + hh * 4 + tl
                        c = hh * 2 * CH + slot * CH + tl * V
                        nc.vector.tensor_scalar(o_pair[:, c:c + V],
                                                o_pair[:, c:c + V],
                                                inv[:, j:j + 1], None, ALU.mult)
                    if last:
                        # final pair: per-half-chunk DMAs, each issued right
                        # after its own 4 scales; the very last transfer is
                        # a quarter of the pair (shorter post-issue drain)
                        nc.sync.dma_start(
                            out_m[:, hh:hh + 1, 4 * ci:4 * ci + 4, :],
                            opr[:, hh:hh + 1, 4 * slot:4 * slot + 4, :])
            if not last:
                # whole pair (2 chunks x both 128-row halves) in ONE DMA
                # issue: the pair covers a contiguous t-range (4D pattern)
                nc.sync.dma_start(out_m[:, :, 4 * c1:4 * c1 + 8, :], opr)

        def emit_tail(ci):
            if not tail_pend:
                stpair = sbst.tile([128, 32], F32, tag="stp", name="stp", bufs=3)
                o_pair = sbo.tile([128, 4 * CH], BF16, tag="o", name="o_pair",
                                  bufs=2)
            else:
                stpair, o_pair = tail_pend[0][1], tail_pend[0][2]
            slot = len(tail_pend)
            tail_st["n"] = tail_st.get("n", 0) + 1
            emit_tail_a(ci, stpair, o_pair, slot, use_act=tail_st["n"] > 3)
            tail_pend.append((ci, stpair, o_pair))
            if len(tail_pend) == 2:
                emit_tail_b(tail_pend)
                tail_pend.clear()

        # software-pipelined emission: stage1(c) | stage2(c-1) | blend(c-2),
        # flowing across step boundaries with no barrier. stage2 of chunk c
        # runs one chunk behind its stage1 so the in-order ACT queue never
        # blocks a ready gelu behind a not-yet-ready tanh.
        work = []   # (step, chunk) in emission order
        for s in range(NEV):
            work += [(s, (s + 1 + j) % NCH) for j in range(NCH)]
        hbuf, tbuf = {}, {}
        for idx, (s, ci) in enumerate(work):
            hbuf[(s, ci)] = emit_stage1(s, ci, MODES[s])
            if idx >= 1:
                s1, c1 = work[idx - 1]
                tbuf[(s1, c1)] = emit_stage2(s1, c1, MODES[s1], hbuf.pop((s1, c1)))
            if idx >= 2:
                s2, c2 = work[idx - 2]
                emit_blend(s2, c2, tbuf.pop((s2, c2)))
            if idx >= 5:
                emit_post(*work[idx - 5])
        for idx in (len(work) - 1,):
            s1, c1 = work[idx]
            tbuf[(s1, c1)] = emit_stage2(s1, c1, MODES[s1], hbuf.pop((s1, c1)))
        for idx in (len(work) - 2, len(work) - 1):
            s2, c2 = work[idx]
            emit_blend(s2, c2, tbuf.pop((s2, c2)))
        for idx in range(len(work) - 5, len(work)):
            emit_post(*work[idx])

    nc.compile()
    return nc


def kernel(**inputs):
    g = {k: np.asarray(v, np.float32) if k != "tokens" else np.asarray(v)
         for k, v in inputs.items()}
    alpha = float(g["alpha"])
    a = float(1.0 / (1.0 + np.exp(-np.float64(alpha))))
    ia = np.float32(1.0 - a)
    ln_b = g["ln_b"]
    has_lnb = bool(np.any(ln_b != 0))
    key = (np.float64(a).tobytes(), has_lnb)
    if key not in _CACHE:
        _CACHE[key] = _build(a, has_lnb)
    nc = _CACHE[key]

    W1, W2 = g["W1"], g["W2"]
    W1c, W1l, W1r, W1b = W1[:D], W1[D:2 * D], W1[2 * D:3 * D], W1[3 * D:]
    embed, pos = g["embed"], g["pos_embed"]
    head_w, ln_g = g["head_w"], g["ln_g"]

    bf = ml_dtypes.bfloat16
    e4 = ml_dtypes.float8_e4m3

    onesb = np.ones((128, 4), np.float32)
    onesb[:, 2] = np.arange(128)
    onesb[:, 3] = np.arange(128, 256)
    onesb = onesb.astype(bf)

    cpack = np.zeros((128, 44), np.float32)
    cpack[:, 0:32] = pos.T * np.float32(1.0 / ia)
    cpack[:, 32:36] = g["c_states"].T
    cpack[:, 36:38] = g["bc1"].reshape(2, 128).T
    cpack[:, 38:39] = g["bc2"].reshape(128, 1)
    cpack[:, 39:41] = g["b1"].reshape(2, 128).T
    cpack[:, 41:42] = g["b2"].reshape(128, 1)
    cpack[:, 42:44] = np.stack([np.arange(128), np.arange(128, 256)], axis=1)

    epack = (np.concatenate([embed[0:128], embed[128:256]], axis=1)
             * np.float32(1.0 / ia)).astype(bf)

    wca = (W1c * np.float32(SC)).astype(np.float32)
    w2f = np.concatenate([W2[0:128], W2[128:256]], axis=1).astype(np.float32)

    ghw = head_w * ln_g[:, None]
    # sqrt(128) factor: the quake-rsqrt runs on the rescaled variance v'
    # (= (var+eps)*128/ia^2), so its output is inv_true * ia / sqrt(128)
    hwc = (ghw - ghw.mean(axis=0, keepdims=True)) * np.float32(np.sqrt(128.0))
    wb = np.zeros((128, 1280), np.float32)
    wb[:, 0:256] = W1c
    wb[:, 256:512] = W1l
    wb[:, 512:768] = W1r
    wb[:, 768:1024] = np.concatenate([W2[0:128], W2[128:256]], axis=1)
    wb[:, 1024:1280] = hwc
    wb = wb.astype(bf)

    w8 = np.zeros((128, 768), np.float32)
    w8[:, 0:128] = W1l[:, 0:128]
    w8[:, 128:256] = W1r[:, 0:128]
    w8[:, 256:384] = W1l[:, 128:256]
    w8[:, 384:512] = W1r[:, 128:256]
    w8[:, 512:640] = W2[0:128]
    w8[:, 640:768] = W2[128:256]
    w8 = np.clip(w8 * np.float32(SC), -240, 240).astype(e4)

    fpack = np.zeros((128, 768), np.float32)
    fpack[:, 0:256] = W1b
    fpack[:, 256:512] = g["Wc1"]
    fpack[:, 512:768] = np.concatenate([g["Wc2"][0:128], g["Wc2"][128:256]], axis=1)

    tokens = g["tokens"]
    in_maps = []
    for c in range(NC):
        tk = tokens[c * BL:(c + 1) * BL].astype(np.float32)   # (BL, T)
        in_maps.append({
            "tok": np.ascontiguousarray(tk.T).reshape(1, NTOK).astype(bf),
            "onesb": onesb, "cpack": cpack, "epack": epack,
            "wca": wca, "w2f": w2f, "wb": wb, "w8": w8, "fpack": fpack,
        })

    kw = {}
    if TRACE:
        kw = dict(trace=True)
    res = run_bass_kernel_spmd(nc, in_maps, core_ids=list(range(NC)), **kw)
    if TRACE and res.exec_time_ns is not None:
        print(f"HW exec time: {res.exec_time_ns} ns")
        kernel.last_exec_ns = res.exec_time_ns
        kernel.last_trace = res.instructions_and_trace
    out = np.stack([np.asarray(res.results[c]["out"], np.float32)
                    for c in range(NC)], axis=0)
    out = out.reshape(B, T, V)
    if has_lnb:
        out = out + (ln_b @ head_w)[None, None, :]
    return np.ascontiguousarray(out)

